# revision 17
# baseline (speedup 1.0000x reference)
"""Tensor-parallel multi-head attention for 8 Trainium2 NeuronCores.

Sharding (TP8 over heads): core c owns heads {2c, 2c+1} (128 q/k/v features)
and computes them for BOTH batch elements; out_proj is column-sharded with
8-core mesh AllGathers of the per-core context shards, split by (local head,
batch) so all but the last overlap remaining attention work; the out-proj
quarters are emitted right after their gathers so they too overlap attention.

Per-core dataflow (activations kept transposed, [feature, token]):
  qT/kT/vT = W.T-chunks @ xT          (PE, bf16, fp32 PSUM accum)
  v        = PE-transpose(vT)          (with an appended ones-column)
  sT[k,q]  = kT-block.T @ qT           (causal: upper-right blocks skipped)
  stage    = copy(sT)                  (DVE PSUM->SBUF, frees the PSUM bank
                                        quickly so the PE can run ahead)
  aT       = exp(stage/8 + mask_bias)  (ACT from SBUF; safe without
                                        max-subtraction: scores ~ N(0,1))
  ctxT;sum = [v|1].T @ aT              (ones row gives the softmax denom)
  ctxT    *= 1/sum                     (per-q-block, as soon as its k-loop
                                        completes)
  AllGather ctxT shard per (head,batch), out-proj quarter per (batch,half)

Attention is emitted as an interleaved stream of two (head, batch) pairs —
each pair processes q-blocks {0,1} (k-chunks 0-7) then {2,3} (k-chunks 0-15)
— so the Tensor engine always has independent matmuls to run while the
Scalar engine works through the exp()s; without this the PE idles in
sub-3.5us slices every k-chunk and the HAM clock gate halves its clock.
Host side only reshapes/concatenates shards (dtype prep of inputs aside).
The gathered feature order is [local-head, core, dh]; the host permutes
wo's input dimension to match.
"""

import sys

for _p in ("/opt/trn_rl_repo",):
    if _p not in sys.path:
        sys.path.append(_p)

import numpy as np
import ml_dtypes

import concourse.bass as bass  # noqa: F401
import concourse.mybir as mybir
import concourse.tile as tile
from concourse import bacc, bass_utils
from concourse.masks import make_identity, make_upper_triangular

BF16 = mybir.dt.bfloat16
F32 = mybir.dt.float32
F32R = mybir.dt.float32r
Exp = mybir.ActivationFunctionType.Exp

B, S, D = 2, 2048, 1024
T = B * S            # 4096 tokens across batches
H, DH = 16, 64
NCORES = 8
HPC = H // NCORES    # heads per core = 2
F = HPC * DH         # features per core = 128
KC = S // 128        # 16 k-chunks per batch
QB = S // 512        # 4 q-blocks of 512 per batch

# attention pair order: (h, b); the last pair's AllGather is split in two
PAIRS = [(0, 0), (1, 0), (0, 1), (1, 1)]

_CACHED = {}


def _build(with_bias: bool):
    nc = bacc.Bacc(
        "TRN2",
        target_bir_lowering=False,
        debug=False,
        enable_asserts=True,
        num_devices=NCORES,
    )
    xT_d = nc.dram_tensor("xT", [D, T], BF16, kind="ExternalInput").ap()
    wqT_d = nc.dram_tensor("wqT", [D, F], BF16, kind="ExternalInput").ap()
    wkT_d = nc.dram_tensor("wkT", [D, F], BF16, kind="ExternalInput").ap()
    wvT_d = nc.dram_tensor("wvT", [D, F], BF16, kind="ExternalInput").ap()
    woT_d = nc.dram_tensor("woT", [D, F], BF16, kind="ExternalInput").ap()
    b_d = {}
    if with_bias:
        for nm in ("bq", "bk", "bv", "bo"):
            b_d[nm] = nc.dram_tensor(nm, [1, F], BF16, kind="ExternalInput").ap()
    maskb_d = nc.dram_tensor("maskb", [128, B * KC], F32, kind="ExternalInput").ap()
    outT_d = nc.dram_tensor("outT", [F, T], F32, kind="ExternalOutput").ap()

    with tile.TileContext(nc) as tc:
        with (
            tc.tile_pool(name="singles", bufs=1) as sg,
            tc.tile_pool(name="att", bufs=4) as att_pool,
            tc.tile_pool(name="stg", bufs=3) as stg_pool,
            tc.tile_pool(name="psA", bufs=2, space="PSUM") as psA,
            tc.tile_pool(name="psB", bufs=4, space="PSUM") as psB,
            tc.tile_pool(name="dram", bufs=1, space="DRAM") as dram,
        ):
            # ---- constants -------------------------------------------------
            ident = sg.tile([128, 128], BF16, name="ident")
            make_identity(nc, ident)
            trimask = sg.tile([128, 128], BF16, name="trimask")
            make_upper_triangular(nc, trimask, val=1.0, diag=True)
            ones64f = sg.tile([1, 64], F32, name="ones64f")
            nc.vector.memset(ones64f, 1.0)
            ones64r = sg.tile([1, 64], F32R, name="ones64r")
            nc.vector.tensor_copy(ones64r, ones64f)
            if with_bias:
                ones512 = sg.tile([1, 512], BF16, name="ones512")
                nc.vector.memset(ones512, 1.0)

            # ---- load inputs (split for early start) -----------------------
            maskb_sb = sg.tile([128, B * KC], F32, name="maskb_sb")
            nc.sync.dma_start(maskb_sb, maskb_d)
            w_sb = {}
            for nm, dd in (("v", wvT_d), ("k", wkT_d), ("q", wqT_d), ("o", woT_d)):
                w_sb[nm] = sg.tile([128, 8, F], BF16, name=f"w{nm}T_sb")
                nc.sync.dma_start(w_sb[nm], dd.rearrange("(o p) f -> p o f", p=128))
            b_sb = {}
            if with_bias:
                for nm in ("bq", "bk", "bv", "bo"):
                    b_sb[nm] = sg.tile([1, F], BF16, name=f"{nm}_sb")
                    nc.sync.dma_start(b_sb[nm], b_d[nm])

            # persistent activations first, xT last (freed first: LIFO stack)
            qT_sb, qT_free = tc.tile([128, T], BF16, name="qT_sb")
            kT_sb, kT_free = tc.tile([128, T], BF16, name="kT_sb")
            ctxT_sb, ctxT_free = tc.tile([64, HPC, T], BF16, name="ctxT_sb")
            vT_sb, vT_free = tc.tile([128, T], BF16, name="vT_sb")
            xT_sb, xT_free = tc.tile([128, 8, T], BF16, name="xT_sb")
            xT_r = xT_d.rearrange("(o p) f -> p o f", p=128)
            for half in range(4):
                for ki in range(8):
                    cs = half * 1024
                    nc.sync.dma_start(
                        xT_sb[:, ki, cs:cs + 1024], xT_r[:, ki, cs:cs + 1024]
                    )

            # ---- projections ----------------------------------------------
            def project(w, bias, dst, which):
                for half in range(4):
                    ps = psA.tile(
                        [128, 1024], F32, tag="work", name=f"p_{which}_{half}"
                    )
                    for nb in range(2):
                        cs = half * 1024 + nb * 512
                        for ki in range(8):
                            nc.tensor.matmul(
                                ps[:, nb * 512:nb * 512 + 512],
                                lhsT=w[:, ki, :],
                                rhs=xT_sb[:, ki, cs:cs + 512],
                                start=(ki == 0),
                                stop=(ki == 7 and not with_bias),
                            )
                        if with_bias:
                            nc.tensor.matmul(
                                ps[:, nb * 512:nb * 512 + 512],
                                lhsT=bias[0:1, :],
                                rhs=ones512[0:1, :],
                                start=False,
                                stop=True,
                            )
                    nc.vector.tensor_copy(
                        dst[:, half * 1024:half * 1024 + 1024], ps
                    )

            project(w_sb["v"], b_sb.get("bv"), vT_sb, "v")
            project(w_sb["k"], b_sb.get("bk"), kT_sb, "k")
            project(w_sb["q"], b_sb.get("bq"), qT_sb, "q")

            # ---- transpose v into [token, feat] blocks with ones column ----
            v_ones = sg.tile([128, B * KC, HPC, DH + 1], BF16, name="v_ones")
            nc.vector.memset(v_ones, 1.0)
            for tb in range(B * KC):
                pt = psB.tile([128, 128], BF16, tag="ctx", name=f"vt_{tb}")
                nc.tensor.transpose(pt, vT_sb[:, tb * 128:tb * 128 + 128], ident)
                for h in range(HPC):
                    nc.vector.tensor_copy(
                        v_ones[:, tb, h, 0:DH], pt[:, h * 64:h * 64 + 64]
                    )

            # xT and vT are no longer needed; free before the attention/out
            # stage so the gathered-context tiles can reuse their space
            xT_free()
            vT_free()

            # ---- attention: interleaved (head, batch) pair streams --------
            sums_r = sg.tile([1, 2 * S], F32R, name="sums_r")
            rec_sb = sg.tile([64, 2048], F32, name="rec_sb")
            outT_sb, outT_free = tc.tile([128, T], F32, name="outT_sb")

            cc_in = {}
            cc_out = {}
            for h, b in PAIRS[:-1]:
                cc_in[(h, b)] = dram.tile([DH, S], BF16, name=f"cci_{h}_{b}")
                cc_out[(h, b)] = dram.tile(
                    [NCORES * DH, S], BF16, addr_space="Shared", name=f"cco_{h}_{b}"
                )
            hL, bL = PAIRS[-1]
            for half in range(2):
                cc_in[(hL, bL, half)] = dram.tile(
                    [DH, 1024], BF16, name=f"cci_L_{half}"
                )
                cc_out[(hL, bL, half)] = dram.tile(
                    [NCORES * DH, 1024], BF16, addr_space="Shared",
                    name=f"cco_L_{half}",
                )
            ctxF = {}
            ctxF_frees = []
            for h in range(HPC):
                for b in range(B):
                    t_, f_ = tc.tile([128, 4, S], BF16, name=f"ctxF_{h}_{b}")
                    ctxF[(h, b)] = t_
                    ctxF_frees.append(f_)

            def normalize_qb(h, b, lane, qb, ctx_tile):
                t0 = b * S
                so = lane * S + qb * 512
                ro = lane * 1024 + (qb % 2) * 512
                nc.vector.tensor_copy(
                    sums_r[0:1, so:so + 512], ctx_tile[DH:DH + 1, :]
                )
                bc = psA.tile(
                    [128, 512], F32, tag="work", name=f"bc_{h}_{b}_{qb}"
                )
                nc.tensor.matmul(
                    bc[0:64, :],
                    lhsT=ones64r[0:1, :],
                    rhs=sums_r[0:1, so:so + 512],
                    start=True,
                    stop=True,
                )
                nc.vector.reciprocal(rec_sb[:, ro:ro + 512], bc[0:64, :])
                nc.vector.tensor_mul(
                    ctxT_sb[:, h, t0 + qb * 512:t0 + qb * 512 + 512],
                    ctx_tile[0:DH, :],
                    rec_sb[:, ro:ro + 512],
                )

            def kc_step(h, b, lane, kc, qlo, qhi, ctx_ps):
                """One k-chunk of one pass: scores -> stage -> exp -> ctx."""
                po = 64 * h
                t0 = b * S
                q0 = kc * 128
                lo = max(q0, qlo)
                w = qhi - lo
                kT_blk = kT_sb[po:po + 64, t0 + q0:t0 + q0 + 128]
                st = psA.tile(
                    [128, 1024], F32, tag="work", name=f"st_{h}_{b}_{kc}_{qlo}"
                )
                c = lo
                while c < qhi:
                    c2 = min(qhi, (c // 512 + 1) * 512)
                    nc.tensor.matmul(
                        st[:, c - qlo:c2 - qlo],
                        lhsT=kT_blk,
                        rhs=qT_sb[po:po + 64, t0 + c:t0 + c2],
                        start=True,
                        stop=True,
                    )
                    c = c2
                sg_t = stg_pool.tile([128, 1024], F32, tag="stg")
                nc.vector.tensor_copy(sg_t[:, 0:w], st[:, lo - qlo:qhi - qlo])
                at = att_pool.tile([128, 1024], BF16, tag="att")
                nc.scalar.activation(
                    at[:, 0:w],
                    sg_t[:, 0:w],
                    Exp,
                    bias=maskb_sb[:, b * KC + kc:b * KC + kc + 1],
                    scale=0.125,
                )
                if lo == q0:  # diagonal 128-block: causal interior
                    nc.vector.tensor_mul(
                        at[:, 0:128], at[:, 0:128], trimask
                    )
                c = lo
                while c < qhi:
                    qb = c // 512
                    c2 = min(qhi, (qb + 1) * 512)
                    nc.tensor.matmul(
                        ctx_ps[qb][0:DH + 1, c - qb * 512:c2 - qb * 512],
                        lhsT=v_ones[:, b * KC + kc, h, :],
                        rhs=at[:, c - lo:c2 - lo],
                        start=(kc == 0),
                        stop=(kc == 4 * qb + 3),
                    )
                    c = c2
                # normalize any q-block whose k-loop just completed
                if kc >= 3 and (kc - 3) % 4 == 0:
                    qb_done = (kc - 3) // 4
                    if qlo <= qb_done * 512 < qhi:
                        normalize_qb(h, b, lane, qb_done, ctx_ps[qb_done])

            def ship(h, b, half=None):
                t0 = b * S
                if half is None:
                    key, cols = (h, b), slice(t0, t0 + S)
                else:
                    key, cols = (h, b, half), slice(
                        t0 + half * 1024, t0 + half * 1024 + 1024
                    )
                nc.sync.dma_start(cc_in[key], ctxT_sb[:, h, cols])
                nc.gpsimd.collective_compute(
                    "AllGather",
                    mybir.AluOpType.bypass,
                    replica_groups=[list(range(NCORES))],
                    ins=[cc_in[key].opt()],
                    outs=[cc_out[key].opt()],
                )
                # stage the gathered chunks into SBUF
                t_ = ctxF[(h, b)]
                if half is None:
                    r = cc_out[key].rearrange("(o p) f -> p o f", p=128)
                    for ki in range(4):
                        nc.sync.dma_start(t_[:, ki, :], r[:, ki, :])
                else:
                    r = cc_out[key].rearrange("(o p) f -> p o f", p=128)
                    for ki in range(4):
                        nc.sync.dma_start(
                            t_[:, ki, half * 1024:half * 1024 + 1024], r[:, ki, :]
                        )

            def outproj_quarter(b, half):
                ps = psA.tile([128, 1024], F32, tag="work", name=f"o_{b}_{half}")
                for nb in range(2):
                    cs = half * 1024 + nb * 512
                    first, last = (0, 0), (HPC - 1, 3)
                    for h in range(HPC):
                        for ki in range(4):
                            nc.tensor.matmul(
                                ps[:, nb * 512:nb * 512 + 512],
                                lhsT=w_sb["o"][:, h * 4 + ki, :],
                                rhs=ctxF[(h, b)][:, ki, cs:cs + 512],
                                start=((h, ki) == first),
                                stop=((h, ki) == last and not with_bias),
                            )
                    if with_bias:
                        nc.tensor.matmul(
                            ps[:, nb * 512:nb * 512 + 512],
                            lhsT=b_sb["bo"][0:1, :],
                            rhs=ones512[0:1, :],
                            start=False,
                            stop=True,
                        )
                cs0 = b * S + half * 1024
                nc.vector.tensor_copy(outT_sb[:, cs0:cs0 + 1024], ps)
                nc.sync.dma_start(
                    outT_d[:, cs0:cs0 + 1024], outT_sb[:, cs0:cs0 + 1024]
                )

            def pair_steps(h, b, lane):
                """Two passes: q-blocks {0,1} over kc 0..7, then {2,3} over
                kc 0..15.  ctx psum tiles are created lazily inside the
                closures via this dict."""
                ctx = {}

                def get_ctx(qb):
                    if qb not in ctx:
                        ctx[qb] = psB.tile(
                            [128, 512], F32, tag="ctx", name=f"cx_{h}_{b}_{qb}"
                        )
                    return ctx[qb]

                class Lazy(dict):
                    def __getitem__(self, qb):
                        return get_ctx(qb)

                lz = Lazy()
                p1 = [
                    (lambda kc=kc: kc_step(h, b, lane, kc, 0, 1024, lz))
                    for kc in range(8)
                ]
                p2 = [
                    (lambda kc=kc: kc_step(h, b, lane, kc, 1024, 2048, lz))
                    for kc in range(KC)
                ]
                return p1, p2

            lanes = {p: i % 2 for i, p in enumerate(PAIRS)}
            steps = {}

            def get_steps(p):
                if p not in steps:
                    steps[p] = pair_steps(p[0], p[1], lanes[p])
                return steps[p]

            # emission schedule: P0.pass1; then interleave Pi.pass2 (16) with
            # P(i+1).pass1 (8) at 2:1; ship AGs at pass completions and emit
            # out-proj quarters as soon as their gathers are in flight.
            for s in get_steps(PAIRS[0])[0]:
                s()
            for i, p in enumerate(PAIRS):
                a2 = get_steps(p)[1]
                nxt = PAIRS[i + 1] if i + 1 < len(PAIRS) else None
                b1 = get_steps(nxt)[0] if nxt is not None else []
                bi = 0
                for j, s in enumerate(a2):
                    s()
                    if j % 2 == 1 and bi < len(b1):
                        b1[bi]()
                        bi += 1
                while bi < len(b1):
                    b1[bi]()
                    bi += 1
                h, b = p
                if i < len(PAIRS) - 1:
                    ship(h, b)
                else:
                    ship(h, b, half=1)
                    outproj_quarter(b, 1)
                if h == 1:  # both heads of batch b gathered (for b == bL,
                    # only its first half at this point)
                    if b != bL:
                        outproj_quarter(b, 0)
                        outproj_quarter(b, 1)
                if nxt is not None and i + 2 == len(PAIRS):
                    # the last pair's pass1 was interleaved above: its first
                    # q-half is complete, ship it early
                    ship(*nxt, half=0)

            # out-proj quarter for the last pair's first half: both heads'
            # first-half gathers exist after ship(hL, bL, half=0) -- but head
            # hL==1 half 0 ships inside the i+2 branch; emit its quarter then
            outproj_quarter(bL, 0)

            for f_ in reversed(ctxF_frees):
                f_()
            outT_free()
            ctxT_free()
            kT_free()
            qT_free()

    nc.compile()
    return nc


def _get_program(with_bias: bool = False):
    key = ("nc", with_bias)
    if key not in _CACHED:
        _CACHED[key] = _build(with_bias)
    return _CACHED[key]


# gathered feature order: [local-head h, core r, dh] -> global feature
# global head of (r, h) is 2r + h, so feature index = (2r + h) * DH + dh
_PERM = np.array(
    [(2 * r + h) * DH + dh for h in range(HPC) for r in range(NCORES) for dh in range(DH)]
)


def kernel(x, mask, wq, bq, wk, bk, wv, bv, wo, bo):
    x = np.asarray(x, dtype=np.float32)
    mask = np.asarray(mask)
    bf = ml_dtypes.bfloat16

    with_bias = any(np.any(np.asarray(bb)) for bb in (bq, bk, bv, bo))
    nc = _get_program(with_bias)

    # [feature, batch*seq] activations
    xT = np.ascontiguousarray(x.reshape(T, D).T).astype(bf)
    maskb = np.ascontiguousarray(
        np.where(np.asarray(mask).reshape(B * KC, 128), -10000.0, 0.0)
        .astype(np.float32)
        .T
    )
    in_maps = []
    for c in range(NCORES):
        fs = slice(c * F, (c + 1) * F)
        m = {
            "xT": xT,
            "wqT": np.ascontiguousarray(np.asarray(wq)[fs, :].T).astype(bf),
            "wkT": np.ascontiguousarray(np.asarray(wk)[fs, :].T).astype(bf),
            "wvT": np.ascontiguousarray(np.asarray(wv)[fs, :].T).astype(bf),
            "woT": np.ascontiguousarray(
                np.asarray(wo)[fs, :].T[_PERM]
            ).astype(bf),
            "maskb": maskb,
        }
        if with_bias:
            m["bq"] = np.asarray(bq)[fs].astype(bf).reshape(1, F)
            m["bk"] = np.asarray(bk)[fs].astype(bf).reshape(1, F)
            m["bv"] = np.asarray(bv)[fs].astype(bf).reshape(1, F)
            m["bo"] = np.asarray(bo)[fs].astype(bf).reshape(1, F)
        in_maps.append(m)

    res = bass_utils.run_bass_kernel_spmd(
        nc, in_maps, core_ids=list(range(NCORES)), trace=False
    )
    _CACHED["last_results"] = res

    out = np.empty((B, S, D), dtype=np.float32)
    for c in range(NCORES):
        o = res.results[c]["outT"]  # [F, T]
        out[:, :, c * F:(c + 1) * F] = o.T.reshape(B, S, F)
    return out


# revision 18
# speedup vs baseline: 1.0820x; 1.0820x over previous
"""Tensor-parallel multi-head attention for 8 Trainium2 NeuronCores.

Sharding (TP8 over heads): core c owns heads {2c, 2c+1} (128 q/k/v features)
and computes them for BOTH batch elements; out_proj is column-sharded with
8-core mesh AllGathers of the per-core context shards, split by (local head,
batch) so all but the last overlap remaining attention work; the out-proj
quarters are emitted right after their gathers so they too overlap attention.

Per-core dataflow (activations kept transposed, [feature, token]):
  qT/kT/vT = W.T-chunks @ xT          (PE, bf16, fp32 PSUM accum)
  v        = PE-transpose(vT)          (with an appended ones-column)
  sT[k,q]  = kT-block.T @ qT           (causal: upper-right blocks skipped)
  aT       = exp(sT/8 + mask_bias)     (ACT from PSUM; safe without
                                        max-subtraction: scores ~ N(0,1))
  ctxT;sum = [v|1].T @ aT              (ones row gives the softmax denom)
  ctxT    *= 1/sum                     (per-q-block, as soon as its k-loop
                                        completes; reciprocal_approx_fast)
  AllGather ctxT shard per (head,batch), out-proj quarter per (batch,half)

Engine queues are FIFO, so emission order is execution order per engine.
The attention stream is software-pipelined: the ctx matmuls of k-chunk step
N are emitted two steps behind its score matmuls, giving the Scalar engine
time to produce exp(scores) before the Tensor engine's queue reaches the
ctx matmul that consumes them.  Two (head, batch) pairs are additionally
interleaved 2:1 so PSUM context accumulators (2 banks per pair under the
two-pass q-block split) fit alongside the double-buffered score tiles.
Host side only reshapes/concatenates shards (dtype prep of inputs aside).
The gathered feature order is [local-head, core, dh]; the host permutes
wo's input dimension to match.
"""

import sys
from collections import deque

for _p in ("/opt/trn_rl_repo",):
    if _p not in sys.path:
        sys.path.append(_p)

import numpy as np
import ml_dtypes

import concourse.bass as bass  # noqa: F401
import concourse.mybir as mybir
import concourse.tile as tile
from concourse import bacc, bass_utils
from concourse.masks import make_identity, make_upper_triangular

BF16 = mybir.dt.bfloat16
F32 = mybir.dt.float32
F32R = mybir.dt.float32r
Exp = mybir.ActivationFunctionType.Exp

B, S, D = 2, 2048, 1024
T = B * S            # 4096 tokens across batches
H, DH = 16, 64
NCORES = 8
HPC = H // NCORES    # heads per core = 2
F = HPC * DH         # features per core = 128
KC = S // 128        # 16 k-chunks per batch
QB = S // 512        # 4 q-blocks of 512 per batch

PAIRS = [(0, 0), (1, 0), (0, 1), (1, 1)]
PIPE_DEPTH = 2       # kc-steps of scores->ctx lag

_CACHED = {}


def _build(with_bias: bool):
    nc = bacc.Bacc(
        "TRN2",
        target_bir_lowering=False,
        debug=False,
        enable_asserts=True,
        num_devices=NCORES,
    )
    xT_d = nc.dram_tensor("xT", [D, T], BF16, kind="ExternalInput").ap()
    wqT_d = nc.dram_tensor("wqT", [D, F], BF16, kind="ExternalInput").ap()
    wkT_d = nc.dram_tensor("wkT", [D, F], BF16, kind="ExternalInput").ap()
    wvT_d = nc.dram_tensor("wvT", [D, F], BF16, kind="ExternalInput").ap()
    woT_d = nc.dram_tensor("woT", [D, F], BF16, kind="ExternalInput").ap()
    b_d = {}
    if with_bias:
        for nm in ("bq", "bk", "bv", "bo"):
            b_d[nm] = nc.dram_tensor(nm, [1, F], BF16, kind="ExternalInput").ap()
    maskb_d = nc.dram_tensor("maskb", [128, B * KC], F32, kind="ExternalInput").ap()
    outT_d = nc.dram_tensor("outT", [F, T], F32, kind="ExternalOutput").ap()

    with tile.TileContext(nc) as tc:
        with (
            tc.tile_pool(name="singles", bufs=1) as sg,
            tc.tile_pool(name="att", bufs=6) as att_pool,
            tc.tile_pool(name="psA", bufs=2, space="PSUM") as psA,
            tc.tile_pool(name="psB", bufs=4, space="PSUM") as psB,
            tc.tile_pool(name="dram", bufs=1, space="DRAM") as dram,
        ):
            # ---- constants -------------------------------------------------
            ident = sg.tile([128, 128], BF16, name="ident")
            make_identity(nc, ident)
            trimask = sg.tile([128, 128], BF16, name="trimask")
            make_upper_triangular(nc, trimask, val=1.0, diag=True)
            ones64f = sg.tile([1, 64], F32, name="ones64f")
            nc.vector.memset(ones64f, 1.0)
            ones64r = sg.tile([1, 64], F32R, name="ones64r")
            nc.vector.tensor_copy(ones64r, ones64f)
            if with_bias:
                ones512 = sg.tile([1, 512], BF16, name="ones512")
                nc.vector.memset(ones512, 1.0)

            # ---- load inputs (split for early start) -----------------------
            maskb_sb = sg.tile([128, B * KC], F32, name="maskb_sb")
            nc.sync.dma_start(maskb_sb, maskb_d)
            w_sb = {}
            for nm, dd in (("v", wvT_d), ("k", wkT_d), ("q", wqT_d), ("o", woT_d)):
                w_sb[nm] = sg.tile([128, 8, F], BF16, name=f"w{nm}T_sb")
                nc.sync.dma_start(w_sb[nm], dd.rearrange("(o p) f -> p o f", p=128))
            b_sb = {}
            if with_bias:
                for nm in ("bq", "bk", "bv", "bo"):
                    b_sb[nm] = sg.tile([1, F], BF16, name=f"{nm}_sb")
                    nc.sync.dma_start(b_sb[nm], b_d[nm])

            # persistent activations first, xT last (freed first: LIFO stack)
            qT_sb, qT_free = tc.tile([128, T], BF16, name="qT_sb")
            kT_sb, kT_free = tc.tile([128, T], BF16, name="kT_sb")
            ctxT_sb, ctxT_free = tc.tile([64, HPC, T], BF16, name="ctxT_sb")
            vT_sb, vT_free = tc.tile([128, T], BF16, name="vT_sb")
            xT_sb, xT_free = tc.tile([128, 8, T], BF16, name="xT_sb")
            xT_r = xT_d.rearrange("(o p) f -> p o f", p=128)
            for half in range(4):
                for ki in range(8):
                    cs = half * 1024
                    nc.sync.dma_start(
                        xT_sb[:, ki, cs:cs + 1024], xT_r[:, ki, cs:cs + 1024]
                    )

            # ---- projections (ki-outer: 4 matmuls per weight load) ---------
            def project(w, bias, dst, which):
                for hp in range(2):
                    pss = [
                        psA.tile(
                            [128, 1024], F32, tag="work",
                            name=f"p_{which}_{2 * hp + i}",
                        )
                        for i in range(2)
                    ]
                    for ki in range(8):
                        for i in range(2):
                            half = 2 * hp + i
                            for nb in range(2):
                                cs = half * 1024 + nb * 512
                                nc.tensor.matmul(
                                    pss[i][:, nb * 512:nb * 512 + 512],
                                    lhsT=w[:, ki, :],
                                    rhs=xT_sb[:, ki, cs:cs + 512],
                                    start=(ki == 0),
                                    stop=(ki == 7 and not with_bias),
                                )
                    for i in range(2):
                        half = 2 * hp + i
                        if with_bias:
                            for nb in range(2):
                                nc.tensor.matmul(
                                    pss[i][:, nb * 512:nb * 512 + 512],
                                    lhsT=bias[0:1, :],
                                    rhs=ones512[0:1, :],
                                    start=False,
                                    stop=True,
                                )
                        nc.vector.tensor_copy(
                            dst[:, half * 1024:half * 1024 + 1024], pss[i]
                        )

            project(w_sb["v"], b_sb.get("bv"), vT_sb, "v")
            project(w_sb["k"], b_sb.get("bk"), kT_sb, "k")
            project(w_sb["q"], b_sb.get("bq"), qT_sb, "q")

            # ---- transpose v into [token, feat] blocks with ones column ----
            v_ones = sg.tile([128, B * KC, HPC, DH + 1], BF16, name="v_ones")
            nc.vector.memset(v_ones, 1.0)
            for tb in range(B * KC):
                pt = psB.tile([128, 128], BF16, tag="ctx", name=f"vt_{tb}")
                nc.tensor.transpose(pt, vT_sb[:, tb * 128:tb * 128 + 128], ident)
                for h in range(HPC):
                    nc.vector.tensor_copy(
                        v_ones[:, tb, h, 0:DH], pt[:, h * 64:h * 64 + 64]
                    )

            xT_free()
            vT_free()

            # ---- attention -------------------------------------------------
            sums_r = sg.tile([1, 2 * S], F32R, name="sums_r")
            rec_sb = sg.tile([64, 2048], F32, name="rec_sb")
            outT_sb, outT_free = tc.tile([128, T], F32, name="outT_sb")

            cc_in = {}
            cc_out = {}
            for h, b in PAIRS[:-1]:
                cc_in[(h, b)] = dram.tile([DH, S], BF16, name=f"cci_{h}_{b}")
                cc_out[(h, b)] = dram.tile(
                    [NCORES * DH, S], BF16, addr_space="Shared", name=f"cco_{h}_{b}"
                )
            hL, bL = PAIRS[-1]
            for half in range(2):
                cc_in[(hL, bL, half)] = dram.tile(
                    [DH, 1024], BF16, name=f"cci_L_{half}"
                )
                cc_out[(hL, bL, half)] = dram.tile(
                    [NCORES * DH, 1024], BF16, addr_space="Shared",
                    name=f"cco_L_{half}",
                )
            ctxF = {}
            ctxF_frees = []
            for h in range(HPC):
                for b in range(B):
                    t_, f_ = tc.tile([128, 4, S], BF16, name=f"ctxF_{h}_{b}")
                    ctxF[(h, b)] = t_
                    ctxF_frees.append(f_)

            def normalize_qb(h, b, lane, qb, ctx_tile):
                t0 = b * S
                so = lane * S + qb * 512
                ro = lane * 1024 + (qb % 2) * 512
                nc.vector.tensor_copy(
                    sums_r[0:1, so:so + 512], ctx_tile[DH:DH + 1, :]
                )
                bc = psA.tile(
                    [128, 512], F32, tag="work", name=f"bc_{h}_{b}_{qb}"
                )
                nc.tensor.matmul(
                    bc[0:64, :],
                    lhsT=ones64r[0:1, :],
                    rhs=sums_r[0:1, so:so + 512],
                    start=True,
                    stop=True,
                )
                nc.vector.reciprocal_approx_fast(
                    rec_sb[:, ro:ro + 512], bc[0:64, :]
                )
                nc.vector.tensor_mul(
                    ctxT_sb[:, h, t0 + qb * 512:t0 + qb * 512 + 512],
                    ctx_tile[0:DH, :],
                    rec_sb[:, ro:ro + 512],
                )

            def scores_part(h, b, kc, qlo, qhi):
                """Emit score matmuls + exp for one k-chunk; returns the
                attention-weights tile for the ctx part."""
                po = 64 * h
                t0 = b * S
                q0 = kc * 128
                lo = max(q0, qlo)
                w = qhi - lo
                kT_blk = kT_sb[po:po + 64, t0 + q0:t0 + q0 + 128]
                st = psA.tile(
                    [128, 1024], F32, tag="work", name=f"st_{h}_{b}_{kc}_{qlo}"
                )
                c = lo
                while c < qhi:
                    c2 = min(qhi, (c // 512 + 1) * 512)
                    nc.tensor.matmul(
                        st[:, c - qlo:c2 - qlo],
                        lhsT=kT_blk,
                        rhs=qT_sb[po:po + 64, t0 + c:t0 + c2],
                        start=True,
                        stop=True,
                    )
                    c = c2
                at = att_pool.tile([128, 1024], BF16, tag="att")
                nc.scalar.activation(
                    at[:, 0:w],
                    st[:, lo - qlo:qhi - qlo],
                    Exp,
                    bias=maskb_sb[:, b * KC + kc:b * KC + kc + 1],
                    scale=0.125,
                )
                if lo == q0:  # diagonal 128-block: causal interior
                    nc.vector.tensor_mul(at[:, 0:128], at[:, 0:128], trimask)
                return at

            def ctx_part(h, b, lane, kc, qlo, qhi, ctx_ps, at):
                t0 = b * S
                q0 = kc * 128
                lo = max(q0, qlo)
                c = lo
                while c < qhi:
                    qb = c // 512
                    c2 = min(qhi, (qb + 1) * 512)
                    nc.tensor.matmul(
                        ctx_ps[qb][0:DH + 1, c - qb * 512:c2 - qb * 512],
                        lhsT=v_ones[:, b * KC + kc, h, :],
                        rhs=at[:, c - lo:c2 - lo],
                        start=(kc == 0),
                        stop=(kc == 4 * qb + 3),
                    )
                    c = c2
                if kc >= 3 and (kc - 3) % 4 == 0:
                    qb_done = (kc - 3) // 4
                    if qlo <= qb_done * 512 < qhi:
                        normalize_qb(h, b, lane, qb_done, ctx_ps[qb_done])

            def ship(h, b, half=None):
                t0 = b * S
                if half is None:
                    key, cols = (h, b), slice(t0, t0 + S)
                else:
                    key, cols = (h, b, half), slice(
                        t0 + half * 1024, t0 + half * 1024 + 1024
                    )
                nc.sync.dma_start(cc_in[key], ctxT_sb[:, h, cols])
                nc.gpsimd.collective_compute(
                    "AllGather",
                    mybir.AluOpType.bypass,
                    replica_groups=[list(range(NCORES))],
                    ins=[cc_in[key].opt()],
                    outs=[cc_out[key].opt()],
                )
                t_ = ctxF[(h, b)]
                r = cc_out[key].rearrange("(o p) f -> p o f", p=128)
                for ki in range(4):
                    if half is None:
                        nc.sync.dma_start(t_[:, ki, :], r[:, ki, :])
                    else:
                        nc.sync.dma_start(
                            t_[:, ki, half * 1024:half * 1024 + 1024], r[:, ki, :]
                        )

            def outproj_quarter(b, half):
                ps = psA.tile([128, 1024], F32, tag="work", name=f"o_{b}_{half}")
                for h in range(HPC):
                    for ki in range(4):
                        for nb in range(2):
                            cs = half * 1024 + nb * 512
                            nc.tensor.matmul(
                                ps[:, nb * 512:nb * 512 + 512],
                                lhsT=w_sb["o"][:, h * 4 + ki, :],
                                rhs=ctxF[(h, b)][:, ki, cs:cs + 512],
                                start=((h, ki) == (0, 0)),
                                stop=((h, ki) == (HPC - 1, 3) and not with_bias),
                            )
                if with_bias:
                    for nb in range(2):
                        nc.tensor.matmul(
                            ps[:, nb * 512:nb * 512 + 512],
                            lhsT=b_sb["bo"][0:1, :],
                            rhs=ones512[0:1, :],
                            start=False,
                            stop=True,
                        )
                cs0 = b * S + half * 1024
                nc.vector.tensor_copy(outT_sb[:, cs0:cs0 + 1024], ps)
                nc.sync.dma_start(
                    outT_d[:, cs0:cs0 + 1024], outT_sb[:, cs0:cs0 + 1024]
                )

            # ---- build the interleaved, software-pipelined stream ---------
            lanes = {p: i % 2 for i, p in enumerate(PAIRS)}
            ctx_tiles = {}

            def get_ctx(p, qb):
                if (p, qb) not in ctx_tiles:
                    ctx_tiles[(p, qb)] = psB.tile(
                        [128, 512], F32, tag="ctx", name=f"cx_{p[0]}_{p[1]}_{qb}"
                    )
                return ctx_tiles[(p, qb)]

            def make_steps(p, pas):
                h, b = p
                qlo, qhi = (0, 1024) if pas == 0 else (1024, 2048)
                kcs = range(8) if pas == 0 else range(KC)
                out = []
                for kc in kcs:
                    out.append((p, kc, qlo, qhi))
                return out

            # stream entries: ("kc", step) | ("flush",) | ("ship", args) |
            #                 ("outproj", b, half)
            stream = []
            stream += [("kc", s) for s in make_steps(PAIRS[0], 0)]
            for i, p in enumerate(PAIRS):
                a2 = make_steps(p, 1)
                nxt = PAIRS[i + 1] if i + 1 < len(PAIRS) else None
                b1 = make_steps(nxt, 0) if nxt is not None else []
                bi = 0
                for j, s in enumerate(a2):
                    stream.append(("kc", s))
                    if j % 2 == 1 and bi < len(b1):
                        stream.append(("kc", b1[bi]))
                        bi += 1
                while bi < len(b1):
                    stream.append(("kc", b1[bi]))
                    bi += 1
                h, b = p
                if i < len(PAIRS) - 1:
                    stream.append(("ship", (h, b, None)))
                else:
                    stream.append(("ship", (h, b, 1)))
                    stream.append(("outproj", (b, 1)))
                if h == 1 and b != bL:
                    stream.append(("outproj", (b, 0)))
                    stream.append(("outproj", (b, 1)))
                if nxt is not None and i + 2 == len(PAIRS):
                    stream.append(("ship", (nxt[0], nxt[1], 0)))
                    stream.append(("outproj", (bL, 0)))

            pending = deque()

            def flush(n=None):
                k = len(pending) if n is None else n
                for _ in range(k):
                    pending.popleft()()

            for kind, arg in stream:
                if kind == "kc":
                    p, kc, qlo, qhi = arg
                    h, b = p
                    at = scores_part(h, b, kc, qlo, qhi)
                    cps = {qb: get_ctx(p, qb) for qb in (qlo // 512, qlo // 512 + 1)}
                    pending.append(
                        lambda h=h, b=b, kc=kc, qlo=qlo, qhi=qhi, cps=cps, at=at:
                        ctx_part(h, b, lanes[(h, b)], kc, qlo, qhi, cps, at)
                    )
                    if len(pending) > PIPE_DEPTH:
                        flush(1)
                elif kind == "ship":
                    flush()
                    h, b, half = arg
                    ship(h, b, half)
                elif kind == "outproj":
                    flush()
                    outproj_quarter(*arg)

            flush()

            for f_ in reversed(ctxF_frees):
                f_()
            outT_free()
            ctxT_free()
            kT_free()
            qT_free()

    nc.compile()
    return nc


def _get_program(with_bias: bool = False):
    key = ("nc", with_bias)
    if key not in _CACHED:
        _CACHED[key] = _build(with_bias)
    return _CACHED[key]


# gathered feature order: [local-head h, core r, dh] -> global feature
# global head of (r, h) is 2r + h, so feature index = (2r + h) * DH + dh
_PERM = np.array(
    [(2 * r + h) * DH + dh for h in range(HPC) for r in range(NCORES) for dh in range(DH)]
)


def kernel(x, mask, wq, bq, wk, bk, wv, bv, wo, bo):
    x = np.asarray(x, dtype=np.float32)
    mask = np.asarray(mask)
    bf = ml_dtypes.bfloat16

    with_bias = any(np.any(np.asarray(bb)) for bb in (bq, bk, bv, bo))
    nc = _get_program(with_bias)

    # [feature, batch*seq] activations
    xT = np.ascontiguousarray(x.reshape(T, D).T).astype(bf)
    maskb = np.ascontiguousarray(
        np.where(np.asarray(mask).reshape(B * KC, 128), -10000.0, 0.0)
        .astype(np.float32)
        .T
    )
    in_maps = []
    for c in range(NCORES):
        fs = slice(c * F, (c + 1) * F)
        m = {
            "xT": xT,
            "wqT": np.ascontiguousarray(np.asarray(wq)[fs, :].T).astype(bf),
            "wkT": np.ascontiguousarray(np.asarray(wk)[fs, :].T).astype(bf),
            "wvT": np.ascontiguousarray(np.asarray(wv)[fs, :].T).astype(bf),
            "woT": np.ascontiguousarray(
                np.asarray(wo)[fs, :].T[_PERM]
            ).astype(bf),
            "maskb": maskb,
        }
        if with_bias:
            m["bq"] = np.asarray(bq)[fs].astype(bf).reshape(1, F)
            m["bk"] = np.asarray(bk)[fs].astype(bf).reshape(1, F)
            m["bv"] = np.asarray(bv)[fs].astype(bf).reshape(1, F)
            m["bo"] = np.asarray(bo)[fs].astype(bf).reshape(1, F)
        in_maps.append(m)

    res = bass_utils.run_bass_kernel_spmd(
        nc, in_maps, core_ids=list(range(NCORES)), trace=False
    )
    _CACHED["last_results"] = res

    out = np.empty((B, S, D), dtype=np.float32)
    for c in range(NCORES):
        o = res.results[c]["outT"]  # [F, T]
        out[:, :, c * F:(c + 1) * F] = o.T.reshape(B, S, F)
    return out
